# revision 3
# baseline (speedup 1.0000x reference)
"""GAT (graph attention) + global mean pool kernel for 8 Trainium2 NeuronCores.

Strategy (dst-sharded edges, no collectives):
  - Host: add self-loops, assign destination nodes to 8*49 blocks of <=128
    nodes (greedy bin-packing balancing edge counts from each half of the
    node-id space), remap edges to (core, block, row), compute per-edge
    attention logits e_raw = a_src[src] + a_dst[dst] (a tiny x @ Wa
    projection), and pack per-core edge tiles of 128.
  - Device (SPMD, 8 cores): build h = x @ W as a bf16 table in HBM (from a
    host-supplied transposed bf16 x), then stream edge tiles: dma_gather h
    rows by edge source, w = exp(leaky_relu(e_raw)), aggregate
    [w*h | w] into per-block PSUM accumulators via selection-matrix
    matmuls, then normalize, mean over heads, +bias, relu, and accumulate
    graph-pooled partial sums with a one-hot matmul.
  - Host: sum the 8 partial pooled tensors, divide by graph sizes, apply the
    final linear layer.

The softmax skips the max-subtraction pass: logits are O(1) for this
problem (asserted on host), so exp() cannot overflow and the result is
mathematically identical to the reference's max-shifted softmax.
"""

import sys

sys.path.insert(0, "/opt/trn_rl_repo/concourse")
sys.path.insert(0, "/opt/trn_rl_repo")

from dataclasses import dataclass

import numpy as np
import ml_dtypes

# ---- problem constants (hardcoded per contest rules) ----
N, E_RAW, IN, H, C, G = 50000, 800000, 128, 8, 32, 64
HC = H * C          # 256
NEG = 0.2           # leaky relu slope
P = 128
NCORES = 8
NBLK = 49           # dst blocks per core
TOTBLK = NCORES * NBLK
NT_TILES = 392      # node tiles in the h table (392*128 = 50176 >= N)
NTP = NT_TILES * P
HALF_TILES = NT_TILES // 2      # 196
HALF_ROWS = HALF_TILES * P      # 25088 (< 2^15, fits int16 indexing)
GBLK = 3            # blocks per dma_gather op

bf16 = ml_dtypes.bfloat16


@dataclass(frozen=True)
class PlanKey:
    ka: int
    kb: int


_COMPILED = {}  # PlanKey -> (nc, names)


# --------------------------------------------------------------------------
# host-side preprocessing
# --------------------------------------------------------------------------

def _pack_blocks(degA, degB):
    """Greedy 2D bin-packing: nodes -> TOTBLK blocks of <=128 nodes,
    balancing per-block edge sums from each source half.
    Returns (node_block[N], node_row[N], ka, kb)."""
    tot = degA + degB
    order = np.argsort(-tot, kind="stable")
    sumA = np.zeros(TOTBLK, np.int64)
    sumB = np.zeros(TOTBLK, np.int64)
    cnt = np.zeros(TOTBLK, np.int64)
    node_block = np.empty(N, np.int32)
    node_row = np.empty(N, np.int32)
    BIG = 1 << 40
    for n in order:
        dA = degA[n]
        dB = degB[n]
        score = np.maximum(sumA + dA, sumB + dB) + (cnt >= P) * BIG
        b = int(np.argmin(score))
        node_block[n] = b
        node_row[n] = cnt[b]
        cnt[b] += 1
        sumA[b] += dA
        sumB[b] += dB
    assert cnt.max() <= P
    ka = int(np.ceil(sumA.max() / P))
    kb = int(np.ceil(sumB.max() / P))
    return node_block, node_row, ka, kb


def _wrap_idx16(flat):
    """Flat index list (len % 128 == 0) -> [128, len//16] int16 wrapped in 16
    partitions and replicated for the 8 gpsimd cores."""
    a = flat.reshape(-1, 16).T.astype(np.int16)
    return np.tile(a, (8, 1))


def preprocess(x, edge_index, batch, W, att_src, att_dst, bias):
    x = np.asarray(x, np.float32)
    ei = np.asarray(edge_index)
    batch = np.asarray(batch).astype(np.int64)
    W = np.asarray(W, np.float32)
    att_src = np.asarray(att_src, np.float32)
    att_dst = np.asarray(att_dst, np.float32)
    bias = np.asarray(bias, np.float32)

    loops = np.arange(N, dtype=np.int64)
    src = np.concatenate([np.asarray(ei[0], np.int64), loops])
    dst = np.concatenate([np.asarray(ei[1], np.int64), loops])
    ET = src.shape[0]

    half = (src >= HALF_ROWS).astype(np.int64)   # 0 = table A, 1 = table B
    degA = np.bincount(dst[half == 0], minlength=N)
    degB = np.bincount(dst[half == 1], minlength=N)
    node_block, node_row, ka, kb = _pack_blocks(degA, degB)

    # per-edge attention logits (host: 0.2% of total flops)
    Wa_s = (W.reshape(IN, H, C) * att_src[None]).sum(-1)   # [IN, H]
    Wa_d = (W.reshape(IN, H, C) * att_dst[None]).sum(-1)
    a_src = x @ Wa_s
    a_dst = x @ Wa_d
    er = a_src[src] + a_dst[dst]                            # [ET, H] f32
    assert np.abs(er).max() < 60.0, "logits too large for exp without max-shift"

    TA = NBLK * ka
    TB = NBLK * kb
    T = TA + TB

    # group edges by (global block, half); position within group -> tile/lane
    key = node_block[dst] * 2 + half
    eorder = np.argsort(key, kind="stable")
    kcnt = np.bincount(key, minlength=TOTBLK * 2)
    starts = np.concatenate([[0], np.cumsum(kcnt)])[:-1]
    pos = np.arange(ET) - starts[key[eorder]]

    es, ed, eh = src[eorder], dst[eorder], half[eorder]
    eb = node_block[ed]
    core = eb // NBLK
    b_in_core = eb % NBLK
    khalf = np.where(eh == 0, ka, kb)
    assert (pos < khalf * P).all()
    tile_local = pos // P
    lane = pos % P
    region0 = np.where(eh == 0, 0, TA)
    t = region0 + b_in_core * np.where(eh == 0, ka, kb) + tile_local

    # per-core upload arrays
    srcrel = np.where(eh == 0, es, es - HALF_ROWS).astype(np.int16)
    flatidx = np.zeros((NCORES, T * P), np.int16)       # pad -> row 0
    dstl = np.zeros((NCORES, P, T), np.float32)
    eraw = np.full((NCORES, P, T, H), -1.0e4, np.float32)
    flatidx[core, t * P + lane] = srcrel
    dstl[core, lane, t] = node_row[ed].astype(np.float32)
    eraw[core, lane, t] = er[eorder]

    # wrapped int16 index arrays, one contiguous column range per gather op
    ngA = -(-NBLK // GBLK)
    idxA = np.zeros((NCORES, P, TA * P // 16), np.int16)
    idxB = np.zeros((NCORES, P, TB * P // 16), np.int16)
    for c in range(NCORES):
        fi = flatidx[c]
        for g in range(ngA):
            nb = min(GBLK, NBLK - g * GBLK)
            a0 = g * GBLK * ka * P
            idxA[c][:, a0 // 16:(a0 + nb * ka * P) // 16] = _wrap_idx16(
                fi[a0:a0 + nb * ka * P])
            b0 = g * GBLK * kb * P
            idxB[c][:, b0 // 16:(b0 + nb * kb * P) // 16] = _wrap_idx16(
                fi[TA * P + b0:TA * P + b0 + nb * kb * P])

    # per-(block,row) graph ids; pads -> G (never matches iota 0..G-1)
    bidx = np.full((NCORES, P, NBLK), float(G), np.float32)
    nb_core = node_block // NBLK
    bidx[nb_core, node_row, node_block % NBLK] = batch.astype(np.float32)

    # transposed bf16 x for the on-device h-table build
    xpad = np.zeros((NTP, IN), np.float32)
    xpad[:N] = x
    xT = np.ascontiguousarray(xpad.T).astype(bf16)          # [128, NTP]

    shared = dict(
        xT=xT,
        wt=W.astype(bf16),                                   # [IN, HC]
        iotaf=np.tile(np.arange(P, dtype=np.float32), (P, 1)),
        biasr=np.tile(bias.astype(np.float32), (P, 1)),      # [P, C]
    )
    per_core = []
    for c in range(NCORES):
        m = dict(shared)
        m.update(
            idxA=idxA[c], idxB=idxB[c],
            dstl=dstl[c],
            eraw=eraw[c].reshape(P, T * H),
            bidx=bidx[c],
        )
        per_core.append(m)

    cntg = np.bincount(batch, minlength=G).astype(np.float32)
    return per_core, PlanKey(ka, kb), cntg


# --------------------------------------------------------------------------
# device program
# --------------------------------------------------------------------------

def build_program(plan: PlanKey):
    from concourse import bacc
    import concourse.mybir as mybir
    import concourse.tile as tile

    ka, kb = plan.ka, plan.kb
    TA = NBLK * ka
    TB = NBLK * kb
    T = TA + TB
    dt = mybir.dt
    f32, bft, i16 = dt.float32, dt.bfloat16, dt.int16
    AX = mybir.AxisListType
    OP = mybir.AluOpType
    ACT = mybir.ActivationFunctionType

    nc = bacc.Bacc("TRN2", debug=False)
    xT = nc.dram_tensor("xT", [P, NTP], bft, kind="ExternalInput")
    wt = nc.dram_tensor("wt", [IN, HC], bft, kind="ExternalInput")
    iotaf = nc.dram_tensor("iotaf", [P, P], f32, kind="ExternalInput")
    biasr = nc.dram_tensor("biasr", [P, C], f32, kind="ExternalInput")
    idxA = nc.dram_tensor("idxA", [P, TA * P // 16], i16, kind="ExternalInput")
    idxB = nc.dram_tensor("idxB", [P, TB * P // 16], i16, kind="ExternalInput")
    dstl = nc.dram_tensor("dstl", [P, T], f32, kind="ExternalInput")
    eraw = nc.dram_tensor("eraw", [P, T * H], f32, kind="ExternalInput")
    bidx = nc.dram_tensor("bidx", [P, NBLK], f32, kind="ExternalInput")
    pout = nc.dram_tensor("pout", [G, C], f32, kind="ExternalOutput")

    tblA = nc.dram_tensor("tblA", [HALF_ROWS, HC], bft)
    tblB = nc.dram_tensor("tblB", [HALF_ROWS, HC], bft)

    with tile.TileContext(nc) as tc:
        with (
            tc.tile_pool(name="const", bufs=1) as cp,
            tc.tile_pool(name="ppool", bufs=1, space="PSUM") as ppl,
        ):
            wt_sb = cp.tile([IN, HC], bft)
            nc.sync.dma_start(wt_sb[:], wt[:, :])
            iotaf_sb = cp.tile([P, P], f32)
            nc.sync.dma_start(iotaf_sb[:], iotaf[:, :])
            biasr_sb = cp.tile([P, C], f32)
            nc.sync.dma_start(biasr_sb[:], biasr[:, :])
            idxA_sb = cp.tile([P, TA * P // 16], i16)
            nc.sync.dma_start(idxA_sb[:], idxA[:, :])
            idxB_sb = cp.tile([P, TB * P // 16], i16)
            nc.sync.dma_start(idxB_sb[:], idxB[:, :])
            dstl_sb = cp.tile([P, T], f32)
            nc.sync.dma_start(dstl_sb[:], dstl[:, :])
            eraw_sb = cp.tile([P, T * H], f32)
            nc.sync.dma_start(eraw_sb[:], eraw[:, :])
            bidx_sb = cp.tile([P, NBLK], f32)
            nc.sync.dma_start(bidx_sb[:], bidx[:, :])

            pooled_ps = ppl.tile([G, C], f32)

            # ---------- phase A: h table build ----------
            BB = 4  # node tiles per staged write (196 % 4 == 0)
            with (
                tc.tile_pool(name="xtp", bufs=1) as xtp,
                tc.tile_pool(name="hb", bufs=3) as hbp,
                tc.tile_pool(name="hbps", bufs=4, space="PSUM") as hpp,
            ):
                xt_sb = xtp.tile([P, NTP], bft, tag="xtsb")
                nc.sync.dma_start(xt_sb[:], xT[:, :])
                for wb in range(NT_TILES // BB):
                    stage = hbp.tile([P, BB * HC], bft, tag="hstage")
                    for k in range(BB):
                        nt = wb * BB + k
                        hps = hpp.tile([P, HC], f32, tag="hps")
                        nc.tensor.matmul(
                            hps[:], xt_sb[:, nt * P:(nt + 1) * P], wt_sb[:],
                            start=True, stop=True)
                        nc.any.tensor_copy(stage[:, k * HC:(k + 1) * HC], hps[:])
                    r0 = wb * BB * P
                    tgt = tblA if r0 < HALF_ROWS else tblB
                    if r0 >= HALF_ROWS:
                        r0 -= HALF_ROWS
                    nc.sync.dma_start(
                        tgt[r0:r0 + BB * P, :].rearrange(
                            "(k p) c -> p k c", p=P),
                        stage[:].rearrange("p (k c) -> p k c", c=HC))

            # ---------- phase B: edge aggregation ----------
            ngA = -(-NBLK // GBLK)
            with (
                tc.tile_pool(name="ep", bufs=2) as ep,
                tc.tile_pool(name="fp", bufs=2) as fp,
                tc.tile_pool(name="aggp", bufs=2, space="PSUM") as aggp,
            ):
                for g in range(ngA):
                    nb = min(GBLK, NBLK - g * GBLK)
                    bufs = {}
                    for (nm, tbl_t, idx_sb, kh, treg) in (
                        ("A", tblA, idxA_sb, ka, 0),
                        ("B", tblB, idxB_sb, kb, TA),
                    ):
                        buf = ep.tile([P, GBLK * kh, HC], bft, tag=f"buf{nm}")
                        L = nb * kh * P
                        c0 = g * GBLK * kh * P // 16
                        nc.gpsimd.dma_gather(
                            out_ap=buf[:, 0:nb * kh, :],
                            in_ap=tbl_t[:, :],
                            idxs_ap=idx_sb[:, c0:c0 + L // 16],
                            num_idxs=L,
                            num_idxs_reg=L,
                            elem_size=HC,
                            single_packet=False,
                        )
                        bufs[nm] = buf

                    for bl in range(nb):
                        b = g * GBLK + bl
                        agg = aggp.tile([P, HC + H], f32, tag="agg")
                        for (nm, kh, treg) in (("A", ka, 0), ("B", kb, TA)):
                            buf = bufs[nm]
                            t0 = treg + b * kh      # global tile index
                            # w = exp(max(e, NEG*e))
                            lr = ep.tile([P, kh * H], f32, tag=f"lr{nm}")
                            esl = eraw_sb[:, t0 * H:(t0 + kh) * H]
                            nc.vector.scalar_tensor_tensor(
                                out=lr[:], in0=esl, scalar=NEG, in1=esl,
                                op0=OP.mult, op1=OP.max)
                            w = ep.tile([P, kh * H], bft, tag=f"w{nm}")
                            nc.scalar.activation(w[:], lr[:], ACT.Exp)
                            # selection matrix per tile
                            sel = ep.tile([P, kh, P], bft, tag=f"sel{nm}")
                            nc.vector.tensor_tensor(
                                out=sel[:],
                                in0=dstl_sb[:, t0:t0 + kh][:, :, None]
                                .to_broadcast([P, kh, P]),
                                in1=iotaf_sb[:, None, :]
                                .to_broadcast([P, kh, P]),
                                op=OP.is_equal)
                            # msg = [w * h | w]
                            msg = ep.tile([P, kh, HC + H], bft, tag=f"msg{nm}")
                            nc.vector.tensor_tensor(
                                out=msg[:, :, 0:HC].rearrange(
                                    "p k (h c) -> p k h c", h=H),
                                in0=buf[:, bl * kh:(bl + 1) * kh, :].rearrange(
                                    "p k (h c) -> p k h c", h=H),
                                in1=w[:].rearrange("p (k h) -> p k h", h=H)
                                [:, :, :, None].to_broadcast([P, kh, H, C]),
                                op=OP.mult)
                            nc.vector.tensor_copy(
                                msg[:, :, HC:HC + H],
                                w[:].rearrange("p (k h) -> p k h", h=H))
                            first = nm == "A"
                            for k in range(kh):
                                nc.tensor.matmul(
                                    agg[:],
                                    sel[:, k, :],
                                    msg[:, k, :],
                                    start=(first and k == 0),
                                    stop=(nm == "B" and k == kh - 1))

                        # ---- finalize block b ----
                        den = fp.tile([P, H], f32, tag="den")
                        nc.vector.tensor_scalar_add(
                            den[:], agg[:, HC:HC + H], 1e-16)
                        rec = fp.tile([P, H], f32, tag="rec")
                        nc.vector.reciprocal(rec[:], den[:])
                        hidw = fp.tile([P, HC], f32, tag="hidw")
                        nc.vector.tensor_tensor(
                            out=hidw[:].rearrange("p (h c) -> p h c", h=H),
                            in0=agg[:, 0:HC].rearrange("p (h c) -> p h c", h=H),
                            in1=rec[:, :, None].to_broadcast([P, H, C]),
                            op=OP.mult)
                        red = fp.tile([P, C], f32, tag="red")
                        nc.vector.tensor_reduce(
                            red[:],
                            hidw[:].rearrange("p (h c) -> p c h", h=H),
                            axis=AX.X, op=OP.add)
                        pre = fp.tile([P, C], f32, tag="pre")
                        nc.vector.scalar_tensor_tensor(
                            out=pre[:], in0=red[:], scalar=1.0 / H,
                            in1=biasr_sb[:], op0=OP.mult, op1=OP.add)
                        hid = fp.tile([P, C], bft, tag="hid")
                        nc.scalar.activation(hid[:], pre[:], ACT.Relu)
                        bsel = fp.tile([P, G], bft, tag="bsel")
                        nc.vector.tensor_tensor(
                            out=bsel[:],
                            in0=bidx_sb[:, b][:, None].to_broadcast([P, G]),
                            in1=iotaf_sb[:, 0:G],
                            op=OP.is_equal)
                        nc.tensor.matmul(
                            pooled_ps[:], bsel[:], hid[:],
                            start=(b == 0), stop=(b == NBLK - 1))

            pooled_sb = cp.tile([G, C], f32)
            nc.vector.tensor_copy(pooled_sb[:], pooled_ps[:])
            nc.sync.dma_start(pout[:, :], pooled_sb[:])

    nc.compile()
    return nc


# --------------------------------------------------------------------------
# entry point
# --------------------------------------------------------------------------

def kernel(x, edge_index, batch, W, att_src, att_dst, bias, lin_w, lin_b):
    from concourse.bass_utils import run_bass_kernel_spmd

    per_core, plan, cntg = preprocess(
        x, edge_index, batch, W, att_src, att_dst, bias)

    if plan not in _COMPILED:
        _COMPILED[plan] = build_program(plan)
    nc = _COMPILED[plan]

    res = run_bass_kernel_spmd(nc, per_core, core_ids=list(range(NCORES)))
    pooled = np.zeros((G, C), np.float64)
    for r in res.results:
        pooled += r["pout"].astype(np.float64)
    pooled = (pooled / np.maximum(cntg, 1.0)[:, None]).astype(np.float32)
    x_t = pooled @ np.asarray(lin_w, np.float32) + np.asarray(lin_b, np.float32)
    return (x_t, pooled)
